# revision 2
# baseline (speedup 1.0000x reference)
"""Self-contained TRN2 Bass kernel for nn_GCL (2-layer GCN + projection),
SPMD across 8 NeuronCores.

  h1 = relu(Ahat @ (x @ W1) + b1)
  h2 = Ahat @ (h1 @ W2) + b2
  out = h2 @ Wp + bp,   Ahat = D^-1/2 (A+I) D^-1/2, deg = indeg(dst)+1

Strategy (graph/data parallel, dst-sharded, chunked-AllGather overlap):
  * Nodes are assigned to 8 cores x 49 blocks of 128 slots by greedy
    load balancing on (indeg+1); self-loops become explicit edges so the
    aggregation is a pure gather+segment-sum.
  * Per layer each core computes zs = dis * (prev @ W) for its slots and
    AllGathers the scaled table in TWO chunks (blocks 0-24, 25-48) so the
    second chunk's transfer overlaps aggregation of the first.
  * Aggregation: merged dma_gather calls pull 512B rows; one-hot S built
    per 128-edge tile via DVE tensor_scalar is_equal (4x mode) feeds
    TensorE matmuls accumulating in PSUM. Chunk-A partials park in SBUF
    (bf16) and are re-injected via an identity matmul in the chunk-B pass.
  * b1 enters as a rank-1 matmul (disinv x b1-row); b2/bp fold into a
    precomputed bpe = b2 @ Wp + bp added at the end.

Compute dtype bf16 (fp32 PSUM accumulation); final output fp32.
"""

from contextlib import ExitStack

import numpy as np
import ml_dtypes

NPBF16 = ml_dtypes.bfloat16

N_NODES, N_EDGES = 50000, 800000
IN_DIM, HID_DIM, OUT_DIM = 512, 256, 256
N_CORES = 8


class _P:
    def __init__(self, tiles_a, tiles_b, gather_queues=4, gbufs=8,
                 call_tiles=8, scratch=16384):
        self.n_nodes = N_NODES
        self.in_dim = IN_DIM
        self.F = HID_DIM
        self.n_cores = N_CORES
        self.npc = N_NODES // N_CORES
        self.bd = 128
        self.blocks = 49
        self.slots = self.blocks * self.bd          # 6272
        self.cA_blocks = 25
        self.cB_blocks = 24
        self.cA_rows = self.cA_blocks * self.bd     # 3200
        self.cB_rows = self.cB_blocks * self.bd     # 3072
        self.tabA_rows = N_CORES * self.cA_rows     # 25600
        self.tabB_rows = N_CORES * self.cB_rows     # 24576
        assert self.tabA_rows < 32768 and self.tabB_rows < 32768
        self.kin = IN_DIM // 128
        self.kf = self.F // 128
        self.gq = gather_queues
        self.gbufs = gbufs
        self.scratch = scratch
        # per-chunk per-block tile counts (uniform across cores)
        self.tiles = [list(tiles_a), list(tiles_b)]   # [2][blocks]
        self.Tmax = max(max(tiles_a), max(tiles_b))
        # gather calls: fixed call_tiles*128 indices each (last call short),
        # streaming over each chunk's tiles in block order. single_packet
        # must be False (packed calls with num_idxs>1024 wedge the device).
        self.call_tiles = call_tiles
        # seg/idx column offsets per (h, b): cumulative tiles (global stream)
        self.tile_off = [[0] * self.blocks, [0] * self.blocks]
        off = 0
        for h in (0, 1):
            for b in range(self.blocks):
                self.tile_off[h][b] = off
                off += self.tiles[h][b]
            if h == 0:
                self.chunkA_tiles = off
        self.total_tiles = off
        self.chunk_base = [0, self.chunkA_tiles]
        self.chunk_tiles = [self.chunkA_tiles,
                            self.total_tiles - self.chunkA_tiles]


def _build_kernel(p, rep=1):
    import concourse.bacc as bacc
    import concourse.mybir as mybir
    import concourse.tile as tile

    BF16, F32, I16 = mybir.dt.bfloat16, mybir.dt.float32, mybir.dt.int16

    nc = bacc.Bacc("TRN2", target_bir_lowering=False, debug=False,
                   num_devices=p.n_cores, num_swdge_queues=p.gq,
                   dynamic_dma_scratch_size=p.scratch)

    xT = nc.dram_tensor("xT", [128, p.blocks * p.kin * 128], BF16,
                        kind="ExternalInput")
    W1 = nc.dram_tensor("W1", [p.in_dim, p.F], BF16, kind="ExternalInput")
    W2 = nc.dram_tensor("W2", [p.F, p.F], BF16, kind="ExternalInput")
    Wp = nc.dram_tensor("Wp", [p.F, p.F], BF16, kind="ExternalInput")
    bper = nc.dram_tensor("bper", [1, p.F], F32, kind="ExternalInput")
    discol = nc.dram_tensor("discol", [128, p.blocks], F32, kind="ExternalInput")
    disinv = nc.dram_tensor("disinv", [1, p.slots], BF16, kind="ExternalInput")
    b1r = nc.dram_tensor("b1r", [1, p.F], BF16, kind="ExternalInput")
    iota = nc.dram_tensor("iota", [128, 128], BF16, kind="ExternalInput")
    ident = nc.dram_tensor("ident", [128, 128], BF16, kind="ExternalInput")
    idx = nc.dram_tensor("idx", [128, p.total_tiles * 8], I16,
                         kind="ExternalInput")
    seg = nc.dram_tensor("seg", [128, p.total_tiles], F32, kind="ExternalInput")
    out = nc.dram_tensor("out", [p.slots, p.F], F32, kind="ExternalOutput")

    with tile.TileContext(nc) as tc, ExitStack() as ctx:
        const = ctx.enter_context(tc.tile_pool(name="const", bufs=1))
        dram = ctx.enter_context(tc.tile_pool(name="dram", bufs=1, space="DRAM"))
        apool = ctx.enter_context(tc.tile_pool(name="acc", bufs=2))
        gpool = ctx.enter_context(tc.tile_pool(name="g", bufs=p.gbufs))
        spool = ctx.enter_context(tc.tile_pool(name="s", bufs=4))
        xpool = ctx.enter_context(tc.tile_pool(name="x", bufs=3))
        hpool = ctx.enter_context(tc.tile_pool(name="h", bufs=4))
        ppool = ctx.enter_context(tc.tile_pool(name="ps", bufs=3, space="PSUM"))
        p2pool = ctx.enter_context(tc.tile_pool(name="ps2", bufs=2, space="PSUM"))

        w1_t = const.tile([128, p.kin * p.F], BF16)
        for k in range(p.kin):
            nc.sync.dma_start(w1_t[:, k * p.F:(k + 1) * p.F],
                              W1[k * 128:(k + 1) * 128, :])
        w2_t = const.tile([128, p.kf * p.F], BF16)
        for k in range(p.kf):
            nc.sync.dma_start(w2_t[:, k * p.F:(k + 1) * p.F],
                              W2[k * 128:(k + 1) * 128, :])
        wp_t = const.tile([128, p.kf * p.F], BF16)
        for k in range(p.kf):
            nc.sync.dma_start(wp_t[:, k * p.F:(k + 1) * p.F],
                              Wp[k * 128:(k + 1) * 128, :])
        bpe_t = const.tile([1, p.F], F32)
        nc.sync.dma_start(bpe_t[:], bper[:])
        onef_t = const.tile([1, 128], F32)
        nc.vector.memset(onef_t[:], 1.0)
        dis_t = const.tile([128, p.blocks], F32)
        nc.sync.dma_start(dis_t[:], discol[:])
        disinv_t = const.tile([1, p.slots], BF16)
        nc.sync.dma_start(disinv_t[:], disinv[:])
        b1r_t = const.tile([1, p.F], BF16)
        nc.sync.dma_start(b1r_t[:], b1r[:])
        iota_t = const.tile([128, 128], BF16)
        nc.sync.dma_start(iota_t[:], iota[:])
        ident_t = const.tile([128, 128], BF16)
        nc.sync.dma_start(ident_t[:], ident[:])
        idx_t = const.tile([128, p.total_tiles * 8], I16)
        nc.sync.dma_start(idx_t[:], idx[:])
        seg_t = const.tile([128, p.total_tiles], F32)
        nc.sync.dma_start(seg_t[:], seg[:])

        zbA = dram.tile([p.cA_rows, p.F], BF16, tag="zbA")
        zbB = dram.tile([p.cB_rows, p.F], BF16, tag="zbB")
        z2bA = dram.tile([p.cA_rows, p.F], BF16, tag="z2bA")
        z2bB = dram.tile([p.cB_rows, p.F], BF16, tag="z2bB")
        tab1A = nc.dram_tensor("tab1A", [p.tabA_rows, p.F], BF16,
                               kind="Internal", addr_space="Shared").ap()
        tab1B = nc.dram_tensor("tab1B", [p.tabB_rows, p.F], BF16,
                               kind="Internal", addr_space="Shared").ap()
        tab2A = nc.dram_tensor("tab2A", [p.tabA_rows, p.F], BF16,
                               kind="Internal", addr_space="Shared").ap()
        tab2B = nc.dram_tensor("tab2B", [p.tabB_rows, p.F], BF16,
                               kind="Internal", addr_space="Shared").ap()

        accs = {1: apool.tile([128, p.blocks * p.F], BF16, tag="acc1",
                              name="acc1"),
                2: apool.tile([128, p.blocks * p.F], BF16, tag="acc2",
                              name="acc2")}

        import concourse.mybir as mybir
        AG = lambda src, dst: nc.gpsimd.collective_compute(
            "AllGather", mybir.AluOpType.bypass,
            replica_groups=[list(range(p.n_cores))],
            ins=[src.opt()], outs=[dst])

        def zwrite(b, zs, bA, bB):
            if b < p.cA_blocks:
                nc.sync.dma_start(bA[b * p.bd:(b + 1) * p.bd, :], zs[:])
            else:
                bb = b - p.cA_blocks
                nc.sync.dma_start(bB[bb * p.bd:(bb + 1) * p.bd, :], zs[:])

        # ---- produce z1 = dis * (x @ W1), chunked AllGather -----------
        def produce_z1():
            for b in range(p.blocks):
                xt = xpool.tile([128, p.kin * 128], BF16, tag="xt")
                nc.sync.dma_start(
                    xt[:], xT[:, b * p.kin * 128:(b + 1) * p.kin * 128])
                ps = p2pool.tile([128, p.F], F32, tag="zps")
                for k in range(p.kin):
                    nc.tensor.matmul(ps[:], xt[:, k * 128:(k + 1) * 128],
                                     w1_t[:, k * p.F:(k + 1) * p.F],
                                     start=(k == 0), stop=(k == p.kin - 1))
                zs = hpool.tile([128, p.F], BF16, tag="zs")
                nc.scalar.activation(zs[:], ps[:],
                                     mybir.ActivationFunctionType.Copy,
                                     scale=dis_t[:, b:b + 1])
                zwrite(b, zs, zbA, zbB)
                if b == p.cA_blocks - 1:
                    AG(zbA, tab1A)
            AG(zbB, tab1B)

        qc = [0]

        def agg_block(h, b, gts, ensure, layer):
            """Matmuls for block b's chunk-h tiles into a PSUM tile.

            h==0: S-matmuls only, last one stops (partial parks in SBUF).
            h==1: opens with identity-reinject of the parked partial, then
            S-matmuls; layer 1 closes with the rank-1 b1-bias matmul, layer 2
            closes on the last S-matmul."""
            nt = p.tiles[h][b]
            toff = p.tile_off[h][b]            # global tile index
            coff = toff - p.chunk_base[h]      # offset within the chunk
            ensure((coff + nt - 1) // p.call_tiles)
            stile = spool.tile([128, p.Tmax, 128], BF16, tag="s")
            iota_b = iota_t[:].rearrange("p f -> p () f").broadcast_to(
                [128, nt, 128])
            seg_b = seg_t[:, toff:toff + nt].rearrange(
                "p t -> p t ()").broadcast_to([128, nt, 128])
            nc.vector.tensor_tensor(stile[:, :nt, :], iota_b, seg_b,
                                    op=mybir.AluOpType.is_equal)

            def rhs(t):
                k, r = divmod(coff + t, p.call_tiles)
                return gts[k][:, r, :]

            ps = ppool.tile([128, p.F], F32, tag="agg")
            if h == 0:
                for t in range(nt):
                    nc.tensor.matmul(ps[:], stile[:, t, :], rhs(t),
                                     start=(t == 0), stop=(t == nt - 1))
            else:
                nc.tensor.matmul(ps[:], ident_t[:],
                                 accs[layer][:, b * p.F:(b + 1) * p.F],
                                 start=True, stop=False)
                bias = layer == 1
                for t in range(nt):
                    nc.tensor.matmul(ps[:], stile[:, t, :], rhs(t),
                                     start=False,
                                     stop=(not bias and t == nt - 1))
                if bias:
                    nc.tensor.matmul(ps[:], disinv_t[:1, b * p.bd:(b + 1) * p.bd],
                                     b1r_t[:1, :], start=False, stop=True)
            return ps

        def make_ensure(h, tab):
            """Returns (gts dict, ensure fn) streaming fixed-size gather
            calls over chunk h's tile stream."""
            gts = {}
            base = p.chunk_base[h]
            TT = p.chunk_tiles[h]

            def ensure(k):
                while k not in gts:
                    knext = max(gts) + 1 if gts else 0
                    ntile = min(p.call_tiles, TT - knext * p.call_tiles)
                    ni = ntile * 128
                    gt = gpool.tile([128, p.call_tiles, p.F], BF16, tag="g")
                    col = (base + knext * p.call_tiles) * 8
                    nc.gpsimd.dma_gather(
                        gt[:, :ntile, :], tab,
                        idx_t[:, col:col + ni // 16],
                        ni, ni, p.F,
                        queue_num=qc[0] % p.gq, single_packet=False)
                    qc[0] += 1
                    gts[knext] = gt
            return gts, ensure

        def transpose_to(src_bf16):
            hT = hpool.tile([128, p.kf * 128], BF16, tag="ht")
            for k in range(p.kf):
                pst = p2pool.tile([128, 128], BF16, tag="tps")
                nc.tensor.transpose(pst[:], src_bf16[:, k * 128:(k + 1) * 128],
                                    ident_t[:])
                nc.scalar.activation(hT[:, k * 128:(k + 1) * 128], pst[:],
                                     mybir.ActivationFunctionType.Copy)
            return hT

        def passA(layer, tab):
            gts, ensure = make_ensure(0, tab)
            for b in range(p.blocks):
                ps = agg_block(0, b, gts, ensure, layer)
                nc.scalar.activation(accs[layer][:, b * p.F:(b + 1) * p.F],
                                     ps[:],
                                     mybir.ActivationFunctionType.Copy)

        def passB(layer, tab):
            gts, ensure = make_ensure(1, tab)
            for b in range(p.blocks):
                ps = agg_block(1, b, gts, ensure, layer)
                if True:
                    if layer == 1:
                        h1 = hpool.tile([128, p.F], BF16, tag="hb")
                        nc.scalar.activation(h1[:], ps[:],
                                             mybir.ActivationFunctionType.Relu,
                                             scale=dis_t[:, b:b + 1])
                        hT = transpose_to(h1)
                        ps2 = p2pool.tile([128, p.F], F32, tag="zps")
                        for k in range(p.kf):
                            nc.tensor.matmul(ps2[:], hT[:, k * 128:(k + 1) * 128],
                                             w2_t[:, k * p.F:(k + 1) * p.F],
                                             start=(k == 0), stop=(k == p.kf - 1))
                        zs = hpool.tile([128, p.F], BF16, tag="zs")
                        nc.scalar.activation(zs[:], ps2[:],
                                             mybir.ActivationFunctionType.Copy,
                                             scale=dis_t[:, b:b + 1])
                        zwrite(b, zs, z2bA, z2bB)
                        if b == p.cA_blocks - 1:
                            AG(z2bA, tab2A)
                    else:
                        h2 = hpool.tile([128, p.F], BF16, tag="hb")
                        nc.scalar.activation(h2[:], ps[:],
                                             mybir.ActivationFunctionType.Copy,
                                             scale=dis_t[:, b:b + 1])
                        hT = transpose_to(h2)
                        ps2 = p2pool.tile([128, p.F], F32, tag="zps")
                        for k in range(p.kf):
                            nc.tensor.matmul(ps2[:], hT[:, k * 128:(k + 1) * 128],
                                             wp_t[:, k * p.F:(k + 1) * p.F],
                                             start=(k == 0), stop=False)
                        nc.tensor.matmul(ps2[:], onef_t[:], bpe_t[:],
                                         start=False, stop=True)
                        of = hpool.tile([128, p.F], F32, tag="of")
                        nc.scalar.activation(of[:], ps2[:],
                                             mybir.ActivationFunctionType.Copy)
                        nc.sync.dma_start(out[b * p.bd:(b + 1) * p.bd, :], of[:])

        for _ in range(rep):
            produce_z1()
            passA(1, tab1A)
            passB(1, tab1B)
            AG(z2bB, tab2B)
            passA(2, tab2A)
            passB(2, tab2B)

    nc.compile()
    return nc


def _wrap_idx(flat):
    NI = flat.shape[0]
    a = np.empty((128, NI // 16), np.int16)
    blk = flat.reshape(NI // 16, 16).T
    for g in range(8):
        a[g * 16:(g + 1) * 16, :] = blk
    return a


def _assign_nodes(indeg):
    """Greedy balanced assignment of nodes to (core, block) bins.

    Returns core_of, block_of, seg_of (position within block)."""
    import heapq
    n = indeg.shape[0]
    nbins = N_CORES * 49
    cap = 128
    load = np.asarray(indeg, np.int64) + 1
    order = np.argsort(-load, kind="stable")
    heap = [(0, b) for b in range(nbins)]
    heapq.heapify(heap)
    counts = np.zeros(nbins, np.int32)
    core_of = np.empty(n, np.int32)
    block_of = np.empty(n, np.int32)
    seg_of = np.empty(n, np.int32)
    for i in order:
        while True:
            l, b = heapq.heappop(heap)
            if counts[b] < cap:
                break
        core_of[i] = b // 49
        block_of[i] = b % 49
        seg_of[i] = counts[b]
        counts[b] += 1
        heapq.heappush(heap, (l + int(load[i]), b))
    return core_of, block_of, seg_of


def _preprocess_graph(edge_index):
    """Everything that depends only on the graph: assignment, buckets,
    per-core idx/seg arrays, tile geometry. Returns (p_args, per_core_meta,
    core_of, slot_of, dis)."""
    src = np.asarray(edge_index[0], np.int64)
    dst = np.asarray(edge_index[1], np.int64)
    n = N_NODES
    indeg = np.bincount(dst, minlength=n)
    deg = indeg.astype(np.float64) + 1.0
    dis = (1.0 / np.sqrt(deg)).astype(np.float32)

    core_of, block_of, seg_of = _assign_nodes(indeg)
    slot_of = block_of * 128 + seg_of

    # self-loops as explicit edges
    allsrc = np.concatenate([src, np.arange(n, dtype=np.int64)])
    alldst = np.concatenate([dst, np.arange(n, dtype=np.int64)])

    cA_rows, cB_rows = 3200, 3072
    s_slot = slot_of[allsrc]
    s_core = core_of[allsrc]
    h = (s_slot >= cA_rows).astype(np.int64)
    tabrow = np.where(h == 0, s_core * cA_rows + s_slot,
                      s_core * cB_rows + (s_slot - cA_rows))
    d_core = core_of[alldst]
    d_block = block_of[alldst]
    d_seg = seg_of[alldst]

    # bucket id: (core, h, block)
    bucket = (d_core * 2 + h) * 49 + d_block
    order = np.lexsort((tabrow, bucket))
    b_sorted = bucket[order]
    r_sorted = tabrow[order]
    g_sorted = d_seg[order]

    nbuckets = N_CORES * 2 * 49
    counts = np.bincount(b_sorted, minlength=nbuckets).reshape(N_CORES, 2, 49)
    starts = np.zeros(nbuckets + 1, np.int64)
    np.cumsum(counts.reshape(-1), out=starts[1:])

    # per-(h, block) tile counts: max over cores
    tiles = np.maximum(1, -(-counts.max(axis=0) // 128))   # [2][49]
    return (tiles, counts, starts, r_sorted, g_sorted,
            core_of, block_of, seg_of, slot_of, dis)


def _preprocess(p, x, edge_index, W1, b1, W2, b2, Wp, bp, graph=None):
    if graph is None:
        graph = _preprocess_graph(edge_index)
    (tiles, counts, starts, r_sorted, g_sorted,
     core_of, block_of, seg_of, slot_of, dis) = graph

    W1b = np.asarray(W1, np.float32).astype(NPBF16)
    W2b = np.asarray(W2, np.float32).astype(NPBF16)
    Wpb = np.asarray(Wp, np.float32).astype(NPBF16)
    bpe = (np.asarray(b2, np.float32) @ np.asarray(Wp, np.float32)
           + np.asarray(bp, np.float32))
    iota = np.broadcast_to(np.arange(128, dtype=np.float32),
                           (128, 128)).astype(NPBF16)
    ident = np.eye(128, dtype=np.float32).astype(NPBF16)

    x = np.asarray(x, np.float32)
    in_maps = []
    for c in range(p.n_cores):
        mine = core_of == c
        # x layout: [128 feat-in-chunk, blocks*kin*128] where col
        # (b*kin + k)*128 + j = x[node at slot b*128+j, k*128+p]
        xl = np.zeros((p.slots, p.in_dim), np.float32)
        xl[slot_of[mine]] = x[mine]
        xT = (xl.reshape(p.blocks, 128, p.kin, 128)
              .transpose(3, 0, 2, 1).reshape(128, -1)).astype(NPBF16)

        disl = np.zeros(p.slots, np.float32)
        disl[slot_of[mine]] = dis[mine]
        discol = np.ascontiguousarray(
            disl.reshape(p.blocks, 128).T).astype(np.float32)
        disinv = np.zeros((1, p.slots), np.float32)
        nz = disl > 0
        disinv[0, nz] = 1.0 / disl[nz]

        idx_cols = []
        seg_cols = []
        for h in (0, 1):
            for b in range(p.blocks):
                bkt = (c * 2 + h) * 49 + b
                s, e = starts[bkt], starts[bkt + 1]
                cap = p.tiles[h][b] * 128
                assert e - s <= cap, (c, h, b, e - s, cap)
                idxp = np.zeros(cap, np.int64)
                segp = np.full(cap, -1.0, np.float32)
                idxp[:e - s] = r_sorted[s:e]
                segp[:e - s] = g_sorted[s:e]
                idx_cols.append(_wrap_idx(idxp.astype(np.int16)))
                seg_cols.append(
                    np.ascontiguousarray(segp.reshape(-1, 128).T))
        idx_sb = np.concatenate(idx_cols, axis=1)
        seg_sb = np.concatenate(seg_cols, axis=1)
        assert idx_sb.shape[1] == p.total_tiles * 8
        assert seg_sb.shape[1] == p.total_tiles

        in_maps.append({
            "xT": xT, "W1": W1b, "W2": W2b, "Wp": Wpb,
            "bper": bpe.reshape(1, p.F).astype(np.float32), "discol": discol,
            "disinv": disinv.astype(NPBF16),
            "b1r": np.asarray(b1, np.float32).reshape(1, p.F).astype(NPBF16),
            "iota": iota, "ident": ident, "idx": idx_sb, "seg": seg_sb,
        })
    return in_maps


_CACHE = {}


def kernel(x, edge_index, W1, b1, W2, b2, Wp, bp):
    """Full inputs in, full output out. Shards across 8 NeuronCores inside."""
    from concourse.bass_utils import run_bass_kernel_spmd

    graph = _preprocess_graph(edge_index)
    tiles = graph[0]
    key = (tuple(tiles[0]), tuple(tiles[1]))
    p = _P(tiles[0], tiles[1])
    if key not in _CACHE:
        _CACHE[key] = _build_kernel(p)
    nc = _CACHE[key]

    in_maps = _preprocess(p, x, edge_index, W1, b1, W2, b2, Wp, bp, graph)
    res = run_bass_kernel_spmd(nc, in_maps, core_ids=list(range(p.n_cores)))
    outs = np.stack([np.asarray(res.results[c]["out"])
                     for c in range(p.n_cores)])
    core_of, slot_of = graph[5], graph[8]
    return outs[core_of, slot_of].astype(np.float32)
